# revision 2
# baseline (speedup 1.0000x reference)
"""Trainium2 Bass kernel for nn_BehavioralCircuit — wavefront v5 (4-stale, tail-ordered).

Sigma(b) computes [h1(b) | h0(b+1)] from bank(b), with
  A1(b)   = x(b).W(b-4)   + S3(b).h1(b-3)   + SD(b).h1(b-2)
          + SM(b).h1(b-1) + K(b).h0(b)
  A0(b+1) = x(b+1).W(b-4) + S4(b+1).h1(b-3) + S3(b+1).h1(b-2)
          + SD(b+1).h1(b-1) + SM(b+1).h0(b)
M(j,i)[tau,t] = u(i)_tau (x(i)_tau . x(j)_t);  SM=dist1, SD=dist2,
S3=dist3, S4=dist4, K=in-block strict-lower.  All fp8, host-built.
The 4-deep staleness lets every non-critical PE op (sbase with a
2-cycle-old W cast, distance>=2 corrections, DW update) run inside the
sigmoid window; the critical path is sigma -> 4 fp8 matmuls -> sigma.
"""

import sys

import numpy as np
import ml_dtypes

sys.path.insert(0, "/opt/trn_rl_repo")

import concourse.bass as bass
import concourse.bacc as bacc
import concourse.tile as tile
from concourse import mybir
from concourse.bass_utils import run_bass_kernel_spmd

TAU = 128
T_FULL = 100000
NB_FULL = (T_FULL + TAU - 1) // TAU      # 782
NH = 512
NCORES = 8
UH = NH // NCORES                        # 64
LR = 0.1
WINDOW = 10
G = 8
EST = 0.0008

F32 = mybir.dt.float32
F16 = mybir.dt.float16
F8 = mybir.dt.float8e4
NP_F8 = ml_dtypes.float8_e4m3
AF = mybir.ActivationFunctionType

KPB = 5 * TAU                            # K|SM|SD|S3|S4


def build_nc(nb: int):
    nc = bacc.Bacc("TRN2", target_bir_lowering=False, debug=False)
    kmm_d = nc.declare_dram_parameter("kmm", [128, nb * KPB], F8, isOutput=False)
    xt_d = nc.declare_dram_parameter("xt", [2, nb * TAU], F16, isOutput=False)
    c_d = nc.declare_dram_parameter("cm", [128, nb * 2], F16, isOutput=False)
    misc_d = nc.declare_dram_parameter("misc", [2, UH + 2], F16, isOutput=False)
    msum_d = nc.declare_dram_parameter("msum", [TAU, nb], F32, isOutput=True)
    with tile.TileContext(nc) as tc:
        _emit(tc, nc, nb, kmm_d, xt_d, c_d, misc_d, msum_d)
    nc.compile()
    return nc


def _emit(tc, nc, nb, kmm_d, xt_d, c_d, misc_d, msum_d):
    from contextlib import ExitStack
    ngroups = (nb + G - 1) // G
    with ExitStack() as ctx:
        singles = ctx.enter_context(tc.tile_pool(name="singles", bufs=1))
        pk = ctx.enter_context(tc.tile_pool(name="kstream", bufs=4))
        px = ctx.enter_context(tc.tile_pool(name="xstream", bufs=4))
        pc = ctx.enter_context(tc.tile_pool(name="cstream", bufs=4))
        ph = ctx.enter_context(tc.tile_pool(name="hbuf", bufs=3))
        pw = ctx.enter_context(tc.tile_pool(name="whbuf", bufs=4))
        psA = ctx.enter_context(tc.tile_pool(name="psA", bufs=6, space="PSUM"))
        psW = ctx.enter_context(tc.tile_pool(name="psW", bufs=1, space="PSUM"))

        misc_sb = singles.tile([2, UH + 2], F16)
        nc.sync.dma_start(out=misc_sb, in_=misc_d[:, :])
        w0_sb = misc_sb[0:2, 0:UH]
        i2_sb = misc_sb[0:2, UH:UH + 2]
        msum_sb = singles.tile([TAU, nb], F32)

        ktiles, xtiles, ctiles = {}, {}, {}

        def load_group(g):
            if g >= ngroups or g in ktiles:
                return
            lo, hi = g * G, min(nb, g * G + G)
            n = hi - lo
            kt = pk.tile([128, G * KPB], F8, tag="k")
            nc.sync.dma_start(out=kt[:, :n * KPB],
                              in_=kmm_d[:, lo * KPB:hi * KPB])
            xt = px.tile([2, G * TAU], F16, tag="x")
            nc.sync.dma_start(out=xt[:, :n * TAU],
                              in_=xt_d[:, lo * TAU:hi * TAU])
            ct = pc.tile([128, G * 2], F16, tag="c")
            nc.sync.dma_start(out=ct[:, :n * 2], in_=c_d[:, lo * 2:hi * 2])
            ktiles[g], xtiles[g], ctiles[g] = kt, xt, ct

        def KMAT(b, k):   # k: 0=K 1=SM 2=SD 3=S3 4=S4
            o = (b % G) * KPB + k * TAU
            return ktiles[b // G][:, o:o + TAU]

        def XT(b):
            return xtiles[b // G][:, (b % G) * TAU:(b % G + 1) * TAU]

        def CM(b):
            return ctiles[b // G][:, (b % G) * 2:(b % G) * 2 + 2]

        def mm(out_ap, lhsT, rhs, start, stop):
            nc.tensor.matmul(out_ap, lhsT=lhsT, rhs=rhs, start=start,
                             stop=stop, skip_group_check=True)

        for g in range(min(3, ngroups)):
            load_group(g)

        # ---- W psum init + wh_init ----
        w_ps = psW.tile([2, UH], F32)
        mm(w_ps, i2_sb, w0_sb, True, False)
        wh_init = pw.tile([2, UH], F16, tag="w")
        nc.vector.tensor_copy(wh_init, w_ps)
        whs = {-2: wh_init, -1: wh_init}

        # ---- prologue ----
        banks = {}
        pro_ps = psA.tile([TAU, 2 * UH], F32, tag="a")
        mm(pro_ps[:, 0:UH], XT(0), wh_init, True, True)
        mm(pro_ps[:, UH:2 * UH], XT(0), wh_init, False, True)
        h_prev = ph.tile([TAU, 2 * UH], F16, tag="h")
        nc.scalar.activation(h_prev, pro_ps, AF.Sigmoid)

        for bb in (0, 1):
            a = psA.tile([TAU, 2 * UH], F32, tag="a")
            mm(a[:, 0:UH], XT(bb), wh_init, True, False)
            if bb + 1 < nb:
                mm(a[:, UH:2 * UH], XT(bb + 1), wh_init, False, False)
            banks[bb] = a

        # ---- main loop ----
        for b in range(nb):
            tc.tile_set_cur_wait(b * EST)
            if b % G == 0:
                load_group(b // G + 3)
            h1s = h_prev[:, 0:UH]        # h1(b-1)
            h0s = h_prev[:, UH:2 * UH]   # h0(b)
            a = banks[b]
            # --- critical: bank(b) writes needing h(b-1)/h0(b) ---
            if b > 0:
                mm(a[:, 0:UH], KMAT(b, 1), h1s, False, False)       # SM(b)
            mm(a[:, 0:UH], KMAT(b, 0), h0s, False, True)            # K(b)
            if b + 1 < nb:
                mm(a[:, UH:2 * UH], KMAT(b + 1, 1), h0s, False,
                   stop=(b == 0))                                   # SM(b+1)
                if b > 0:
                    mm(a[:, UH:2 * UH], KMAT(b + 1, 2), h1s, False,
                       True)                                        # SD(b+1)
            # --- sigma(b) ---
            h_cur = ph.tile([TAU, 2 * UH], F16, tag="h")
            if b + 1 < nb:
                nc.scalar.activation(h_cur, a, AF.Sigmoid)
            else:
                nc.scalar.activation(h_cur[:, 0:UH], a[:, 0:UH], AF.Sigmoid)
            # --- slack into bank(b+1) (rhs h1(b-1)), shared LDW ---
            if b > 0:
                if b + 1 < nb:
                    mm(banks[b + 1][:, 0:UH], KMAT(b + 1, 2), h1s,
                       False, False)                                # SD(b+1)
                if b + 2 < nb:
                    mm(banks[b + 1][:, UH:2 * UH], KMAT(b + 2, 3), h1s,
                       False, False)                                # S3(b+2)
            # --- deferred: DW(b-1), wh cast, msum(b-1) ---
            if b > 0:
                mm(w_ps, CM(b - 1), h1s, False, b == nb - 1)
                wh_b = pw.tile([2, UH], F16, tag="w")
                nc.vector.tensor_copy(wh_b, w_ps)
                whs[b - 1] = wh_b
                nc.vector.reduce_sum(msum_sb[:, b - 1:b], h1s,
                                     axis=mybir.AxisListType.X)
            # --- sbase for bank(b+2), rhs = wh(b-2): CAST'd a full
            # cycle back in PE program order -> never stalls ---
            if b + 2 < nb:
                a2 = psA.tile([TAU, 2 * UH], F32, tag="a")
                mm(a2[:, 0:UH], XT(b + 2), whs[b - 2], True, False)
                if b + 3 < nb:
                    mm(a2[:, UH:2 * UH], XT(b + 3), whs[b - 2], False, False)
                banks[b + 2] = a2
                # slack into the fresh bank(b+2), after its sbase
                if b > 0:
                    mm(banks[b + 2][:, 0:UH], KMAT(b + 2, 3), h1s,
                       False, False)                                # S3(b+2)
                    if b + 3 < nb:
                        mm(banks[b + 2][:, UH:2 * UH], KMAT(b + 3, 4), h1s,
                           False, False)                            # S4(b+3)
            h_prev = h_cur
            banks.pop(b, None)
            whs.pop(b - 4, None)

        nc.vector.reduce_sum(msum_sb[:, nb - 1:nb], h_prev[:, 0:UH],
                             axis=mybir.AxisListType.X)
        nc.sync.dma_start(out=msum_d[:, :], in_=msum_sb)


# ---------------------------------------------------------------------------

def _host_prep(X, rewards, nb):
    tpad = nb * TAU
    t_real = min(T_FULL, tpad, X.shape[0])
    X = X[:t_real]
    rewards = rewards[:t_real]
    Xp = np.zeros((tpad, 2), np.float32)
    Xp[:t_real] = X
    cs = np.cumsum(rewards.astype(np.float64))
    sh = np.concatenate([np.zeros(WINDOW), cs[:-WINDOW]])
    cnt = np.minimum(np.arange(t_real) + 1.0, float(WINDOW))
    r = rewards.astype(np.float64) - (cs - sh) / cnt
    u = np.zeros(tpad, np.float32)
    u[:t_real] = (LR * r).astype(np.float32)

    Xb = Xp.reshape(nb, TAU, 2)
    ub = u.reshape(nb, TAU)

    mask = np.triu(np.ones((TAU, TAU), np.float32), 1)
    pack = np.zeros((nb, TAU, KPB), np.float32)
    pack[:, :, 0:TAU] = np.einsum('bic,bjc->bij', Xb, Xb) \
        * ub[:, :, None] * mask[None]
    for d in (1, 2, 3, 4):
        gm = np.einsum('bic,bjc->bij', Xb[:-d], Xb[d:])
        pack[d:, :, d * TAU:(d + 1) * TAU] = gm * ub[:-d, :, None]
    kmm = np.ascontiguousarray(
        pack.transpose(1, 0, 2).reshape(TAU, nb * KPB)).astype(NP_F8)

    xt = np.ascontiguousarray(Xp.T).astype(np.float16)
    cm = np.ascontiguousarray(
        (ub[:, :, None] * Xb).transpose(1, 0, 2).reshape(TAU, nb * 2)
    ).astype(np.float16)
    return kmm, xt, cm


def run_cores(X, rewards, W0, nb=NB_FULL, trace=False):
    kmm, xt, cm = _host_prep(X, rewards, nb)
    nc = build_nc(nb)
    in_maps = []
    for c in range(NCORES):
        w0c = W0[c * UH:(c + 1) * UH]
        misc = np.zeros((2, UH + 2), np.float16)
        misc[:, :UH] = w0c.T.astype(np.float16)
        misc[:, UH:UH + 2] = np.eye(2, dtype=np.float16)
        in_maps.append({"kmm": kmm, "xt": xt, "cm": cm, "misc": misc})
    res = run_bass_kernel_spmd(nc, in_maps, list(range(NCORES)), trace=trace)
    msums = [res.results[c]["msum"] for c in range(NCORES)]
    total = np.sum(msums, axis=0) / float(NH)
    m = total.T.reshape(-1)[:T_FULL].astype(np.float32)
    return m, res


def kernel(X, rewards, W_plastic_init):
    m, _ = run_cores(np.asarray(X, np.float32),
                     np.asarray(rewards, np.float32),
                     np.asarray(W_plastic_init, np.float32))
    return m
